# revision 16
# baseline (speedup 1.0000x reference)
"""APPNP block (10-hop propagation + FFN) on 8 TRN2 NeuronCores.

Strategy:
- Nodes sharded across 8 cores by dst block (12500 real + 44 pad = 12544 each),
  per-core relabeled by descending in-degree.
- Per hop: each core publishes its normalized state block via AllGather into a
  replicated fp32 table, then gathers its in-edges' source rows with chunked
  dma_gather calls. The table is viewed as 512B rows holding a PAIR of nodes,
  so signed int16 offsets (with a per-call base) reach the whole 100352-node
  space in one grid; a predicated copy selects the correct half. Slot planes
  are then reduced on the Vector engine (no scatter-add: avoids CCE RMW races
  between duplicate destination rows).
- FFN runs on-device (PE transposes + matmuls); host inverts the relabeling.
"""
import os
import sys

import numpy as np

sys.path.insert(0, "/opt/trn_rl_repo")

N_NODES = 100000
N_EDGES = 1600000
D = 48
DP = 64                # padded fp32 row (256B); pair row = 128 (512B)
ALPHA = 0.1
HOPS = int(os.environ.get("KERNEL_HOPS", "10"))
NC_N = 8
NLOC_REAL = 12500
NLOC = 12544           # 98 * 128
NCOL = NLOC // 128     # 98
NTOT = NC_N * NLOC     # 100352
NPAIR = NTOT // 2      # 50176 pair rows
WMAX = 7               # columns per gather call (7*128+1 = 897 idxs <= 1024)
CALL_COLS_OF = lambda w: (w * 128 + 1 + 15) // 16
MSG_COLS = 8

# pair rows whose both halves are always zero (pad nodes of each rank)
ZPAIRS = np.array([(12544 * r + 12500) // 2 for r in range(NC_N)], np.int64)

LAST_EXEC_NS = None


def _build_host_structures(src, dst):
    deg = np.bincount(dst, minlength=N_NODES)

    owner = dst // NLOC_REAL
    core_edges = [np.where(owner == c)[0] for c in range(NC_N)]

    perms, inv_perms = [], []
    for c in range(NC_N):
        lo = c * NLOC_REAL
        p = np.argsort(-deg[lo:lo + NLOC_REAL], kind="stable")
        perms.append(p)
        ip = np.empty(NLOC_REAL, np.int64)
        ip[p] = np.arange(NLOC_REAL)
        inv_perms.append(ip)

    # table row of each edge's source node
    so = src // NLOC_REAL
    src_local = src - so * NLOC_REAL
    offs = np.empty_like(src_local)
    for c in range(NC_N):
        m = so == c
        offs[m] = inv_perms[c][src_local[m]]
    src_row = so * NLOC + offs

    # unified per-column slot counts (deg-sorted => non-increasing per core)
    K = np.zeros(NCOL, np.int64)
    for c in range(NC_N):
        lo = c * NLOC_REAL
        s0 = np.zeros(NLOC, np.int64)
        s0[:NLOC_REAL] = deg[lo:lo + NLOC_REAL][perms[c]]
        np.maximum(K, s0.reshape(NCOL, 128).max(1), out=K)

    # call plan: per slot s, prefix of W_s columns, chunked by WMAX
    calls = []  # (slot, col_start, w)
    smax = int(K.max())
    for s in range(smax):
        Ws = int((K > s).sum())
        q = 0
        while q < Ws:
            w = min(WMAX, Ws - q)
            calls.append((s, q, w))
            q += w

    totcols = sum(CALL_COLS_OF(w) for _, _, w in calls)
    sumw = sum(w for _, _, w in calls)

    # per-core dense (slot, node) -> src_row maps
    grid_maps = []
    for c in range(NC_N):
        e = core_edges[c]
        d_pos = inv_perms[c][dst[e] - c * NLOC_REAL]
        rows = src_row[e]
        order = np.argsort(d_pos, kind="stable")
        rows = rows[order]
        ep = d_pos[order]
        slot = np.zeros(len(ep), np.int64)
        if len(ep):
            starts = np.r_[0, np.where(np.diff(ep) != 0)[0] + 1]
            cnt = np.diff(np.r_[starts, len(ep)])
            slot = np.arange(len(ep)) - np.repeat(starts, cnt)
        gm = np.full((smax, NLOC), -1, np.int64)
        gm[slot, ep] = rows
        grid_maps.append(gm)

    # per-call base: max pair-row over all cores minus int16 headroom
    bases = []
    for (s, cst, w) in calls:
        mx = 0
        for c in range(NC_N):
            rr = grid_maps[c][s, cst * 128:(cst + w) * 128]
            rv = rr[rr >= 0]
            if len(rv):
                mx = max(mx, int(rv.max()) >> 1)
        bases.append(max(0, mx - 32767))

    gidx = np.zeros((NC_N, 128, totcols), np.int16)
    selm = np.zeros((NC_N, 128, sumw), np.int8)

    for c in range(NC_N):
        gm = grid_maps[c]
        col = 0
        mcol = 0
        for ci, (s, cst, w) in enumerate(calls):
            base = bases[ci]
            ni = w * 128 + 1
            L = CALL_COLS_OF(w)
            rr = gm[s, cst * 128:(cst + w) * 128]
            real = rr >= 0
            zp = int(ZPAIRS[np.searchsorted(ZPAIRS, base)])  # zero pair >= base
            r2 = np.where(real, rr >> 1, zp)
            sel = np.where(real, rr & 1, 0)
            off = r2 - base
            assert off.min() >= -32768 and off.max() <= 32767

            vals = np.full(L * 16, zp - base, np.int64)
            vals[:ni - 1] = off
            vals[ni - 1] = zp - base   # trailing non-negative dummy
            wrapped = np.empty((16, L), np.int64)
            ii = np.arange(L * 16)
            wrapped[ii % 16, ii // 16] = vals
            gidx[c][:, col:col + L] = np.tile(wrapped.astype(np.int16), (8, 1))
            selm[c][:, mcol:mcol + w] = sel.reshape(w, 128).T
            col += L
            mcol += w

    return {
        "deg": deg, "perms": perms, "calls": calls, "bases": bases,
        "totcols": totcols, "sumw": sumw, "gidx": gidx, "selm": selm,
    }


_BUILD_CACHE = {}


def _build_program(calls, bases, totcols, sumw):
    key = (tuple(calls), tuple(bases))
    if key in _BUILD_CACHE:
        return _BUILD_CACHE[key]

    import concourse.bacc as bacc
    import concourse.bass as bass
    import concourse.tile as tile
    from concourse import mybir
    from concourse.masks import make_identity

    f32 = mybir.dt.float32
    i16 = mybir.dt.int16

    nc = bacc.Bacc("TRN2", target_bir_lowering=False, debug=False,
                   num_devices=NC_N, num_swdge_queues=4)

    ginit_d = nc.dram_tensor("ginit", [NLOC, DP], f32, kind="ExternalInput")
    g0s_d = nc.dram_tensor("g0s", [NLOC, D], f32, kind="ExternalInput")
    n2s_d = nc.dram_tensor("n2s", [NLOC, 1], f32, kind="ExternalInput")
    inv_d = nc.dram_tensor("invn", [NLOC, 1], f32, kind="ExternalInput")
    featT_d = nc.dram_tensor("featT", [D, NLOC], f32, kind="ExternalInput")
    w1_d = nc.dram_tensor("w1", [D, D], f32, kind="ExternalInput")
    w2_d = nc.dram_tensor("w2", [D, D], f32, kind="ExternalInput")
    b1_d = nc.dram_tensor("b1", [D, 1], f32, kind="ExternalInput")
    b2_d = nc.dram_tensor("b2", [D, 1], f32, kind="ExternalInput")
    gidx_d = nc.dram_tensor("gidx", [128, totcols], i16, kind="ExternalInput")
    selm_d = nc.dram_tensor("selm", [128, sumw], mybir.dt.int8, kind="ExternalInput")

    r_out = nc.dram_tensor("r_out", [NLOC, D], f32, kind="ExternalOutput")
    rst_out = nc.dram_tensor("rst_out", [NLOC, D], f32, kind="ExternalOutput")

    bf16 = mybir.dt.bfloat16
    table = nc.dram_tensor("gtable", [NPAIR, 2 * DP], bf16, addr_space="Shared")
    bounce = nc.dram_tensor("gbounce", [NLOC, DP], bf16)

    with tile.TileContext(nc) as tc:
        with tc.tile_pool(name="persist", bufs=1) as pp, \
             tc.tile_pool(name="msgs", bufs=20) as mp, \
             tc.tile_pool(name="psum", bufs=2, space="PSUM") as psp:

            gix = pp.tile([128, totcols], i16, name="gix")
            nc.sync.dma_start(out=gix[:], in_=gidx_d[:, :])
            msk = pp.tile([128, sumw], mybir.dt.int8, name="msk")
            nc.sync.dma_start(out=msk[:], in_=selm_d[:, :])

            g = pp.tile([128, NCOL, DP], f32, name="g")
            nc.sync.dma_start(out=g[:], in_=ginit_d.ap().rearrange("(c p) f -> p c f", p=128))
            g0s = pp.tile([128, NCOL, D], f32, name="g0s")
            nc.sync.dma_start(out=g0s[:], in_=g0s_d.ap().rearrange("(c p) f -> p c f", p=128))
            n2s = pp.tile([128, NCOL, 1], f32, name="n2s")
            nc.sync.dma_start(out=n2s[:], in_=n2s_d.ap().rearrange("(c p) f -> p c f", p=128))

            agg = pp.tile([128, NCOL, D], f32, name="agg")
            gb = pp.tile([128, NCOL, DP], mybir.dt.bfloat16, name="gb")

            for hop in range(HOPS):
                nc.vector.tensor_copy(out=gb[:], in_=g[:])
                nc.sync.dma_start(out=bounce.ap().rearrange("(c p) f -> p c f", p=128), in_=gb[:])
                nc.gpsimd.collective_compute(
                    "AllGather", mybir.AluOpType.bypass,
                    replica_groups=[list(range(NC_N))],
                    ins=[bounce.ap().opt()], outs=[table.ap().opt()],
                )
                nc.vector.memset(agg[:], 0.0)

                qn = 0
                col = 0
                mcol = 0
                for ci, (s, cst, w) in enumerate(calls):
                    base = bases[ci]
                    ni = w * 128 + 1
                    L = CALL_COLS_OF(w)
                    msg = mp.tile([128, MSG_COLS, 2 * DP], mybir.dt.bfloat16, tag="msg",
                                  name=f"msg_{hop}_{ci}")
                    nc.gpsimd.dma_gather(
                        out_ap=msg[:, :(ni + 127) // 128, :],
                        in_ap=table[base:base + 128, :],
                        idxs_ap=gix[:, col:col + L],
                        num_idxs=ni,
                        num_idxs_reg=ni,
                        elem_size=2 * DP,
                        elem_step=2 * DP,
                        queue_num=qn,
                    )
                    qn = (qn + 1) % 4
                    # fold pair halves in place: lo = sel ? hi : lo
                    nc.vector.copy_predicated(
                        out=msg[:, :w, 0:D],
                        mask=msk[:, mcol:mcol + w].rearrange("p (w u) -> p w u", u=1)
                            .to_broadcast([128, w, D]),
                        data=msg[:, :w, DP:DP + D],
                    )
                    nc.vector.tensor_tensor(
                        out=agg[:, cst:cst + w, :],
                        in0=agg[:, cst:cst + w, :],
                        in1=msg[:, :w, 0:D],
                        op=mybir.AluOpType.add,
                    )
                    col += L
                    mcol += w

                nc.vector.tensor_tensor(out=agg[:], in0=agg[:],
                                        in1=n2s[:].to_broadcast([128, NCOL, D]),
                                        op=mybir.AluOpType.mult)
                nc.vector.tensor_tensor(out=g[:, :, :D], in0=agg[:], in1=g0s[:],
                                        op=mybir.AluOpType.add)

            # r = inv * g (reuse agg as h10)
            inv = pp.tile([128, NCOL, 1], f32, name="inv")
            nc.sync.dma_start(out=inv[:], in_=inv_d.ap().rearrange("(c p) f -> p c f", p=128))
            nc.vector.tensor_tensor(out=agg[:], in0=g[:, :, :D],
                                    in1=inv[:].to_broadcast([128, NCOL, D]),
                                    op=mybir.AluOpType.mult)
            h10 = agg
            nc.sync.dma_start(out=r_out.ap().rearrange("(c p) f -> p c f", p=128), in_=h10[:])

            # ---- FFN, chunked over node columns ----
            ident = pp.tile([128, 128], f32, name="ident")
            make_identity(nc, ident)
            w1t = pp.tile([D, D], f32, name="w1t")
            nc.sync.dma_start(out=w1t[:], in_=w1_d[:, :])
            w2t = pp.tile([D, D], f32, name="w2t")
            nc.sync.dma_start(out=w2t[:], in_=w2_d[:, :])
            b1t = pp.tile([D, 1], f32, name="b1t")
            nc.sync.dma_start(out=b1t[:], in_=b1_d[:, :])
            b2t = pp.tile([D, 1], f32, name="b2t")
            nc.sync.dma_start(out=b2t[:], in_=b2_d[:, :])

            CHC = 4
            with tc.tile_pool(name="ffnc", bufs=3) as fc:
                q = 0
                while q < NCOL:
                    w = min(CHC, NCOL - q)
                    nn = w * 128
                    hTc = fc.tile([D, CHC * 128], f32, tag="hTc", name=f"hTc{q}")
                    for c in range(w):
                        pt = psp.tile([D, 128], f32, tag="pt", name=f"pt{q}_{c}")
                        nc.tensor.transpose(out=pt[:], in_=h10[:, q + c, :], identity=ident[:])
                        nc.scalar.copy(out=hTc[:, c * 128:(c + 1) * 128], in_=pt[:])
                    pm = psp.tile([D, CHC * 128], f32, tag="pm", name=f"pm{q}")
                    nc.tensor.matmul(out=pm[:, :nn], lhsT=w1t[:], rhs=hTc[:, :nn],
                                     start=True, stop=True)
                    ff1c = fc.tile([D, CHC * 128], f32, tag="ff1c", name=f"ff1c{q}")
                    nc.vector.tensor_tensor(out=ff1c[:, :nn], in0=pm[:, :nn],
                                            in1=b1t[:].to_broadcast([D, nn]),
                                            op=mybir.AluOpType.add)
                    nc.vector.tensor_scalar_max(out=ff1c[:, :nn], in0=ff1c[:, :nn], scalar1=0.0)
                    pm2 = psp.tile([D, CHC * 128], f32, tag="pm2", name=f"pm2{q}")
                    nc.tensor.matmul(out=pm2[:, :nn], lhsT=w2t[:], rhs=ff1c[:, :nn],
                                     start=True, stop=True)
                    fTc = fc.tile([D, CHC * 128], f32, tag="fTc", name=f"fTc{q}")
                    nc.sync.dma_start(out=fTc[:, :nn], in_=featT_d[:, q * 128:(q * 128 + nn)])
                    rTc = fc.tile([D, CHC * 128], f32, tag="rTc", name=f"rTc{q}")
                    nc.vector.tensor_tensor(out=rTc[:, :nn], in0=pm2[:, :nn],
                                            in1=fTc[:, :nn], op=mybir.AluOpType.add)
                    nc.vector.tensor_tensor(out=rTc[:, :nn], in0=rTc[:, :nn],
                                            in1=b2t[:].to_broadcast([D, nn]),
                                            op=mybir.AluOpType.add)
                    rc = fc.tile([128, CHC, D], f32, tag="rc", name=f"rc{q}")
                    for c in range(w):
                        pb = psp.tile([128, D], f32, tag="pb", name=f"pb{q}_{c}")
                        nc.tensor.transpose(out=pb[:], in_=rTc[:, c * 128:(c + 1) * 128],
                                            identity=ident[:D, :D])
                        nc.scalar.copy(out=rc[:, c, :], in_=pb[:])
                    nc.sync.dma_start(
                        out=rst_out.ap().rearrange("(c p) f -> p c f", p=128)[:, q:q + w, :],
                        in_=rc[:, :w, :])
                    q += w

    nc.compile()
    _BUILD_CACHE[key] = nc
    return nc


def kernel(features, src, dst, w1, b1, w2, b2):
    global LAST_EXEC_NS
    features = np.asarray(features, np.float32)
    src = np.asarray(src).astype(np.int64)
    dst = np.asarray(dst).astype(np.int64)
    w1 = np.asarray(w1, np.float32)
    b1 = np.asarray(b1, np.float32)
    w2 = np.asarray(w2, np.float32)
    b2 = np.asarray(b2, np.float32)

    H = _build_host_structures(src, dst)
    deg, perms = H["deg"], H["perms"]

    norm = (1.0 / np.sqrt(np.maximum(deg, 1.0))).astype(np.float32)

    in_maps = []
    for c in range(NC_N):
        lo = c * NLOC_REAL
        p = perms[c]
        feat_c = features[lo:lo + NLOC_REAL][p]
        norm_c = norm[lo:lo + NLOC_REAL][p]

        ginit = np.zeros((NLOC, DP), np.float32)
        ginit[:NLOC_REAL, :D] = feat_c * norm_c[:, None]
        g0s = np.zeros((NLOC, D), np.float32)
        g0s[:NLOC_REAL] = ALPHA * ginit[:NLOC_REAL, :D]
        n2s = np.zeros((NLOC, 1), np.float32)
        n2s[:NLOC_REAL, 0] = (1.0 - ALPHA) * norm_c * norm_c
        invn = np.zeros((NLOC, 1), np.float32)
        invn[:NLOC_REAL, 0] = 1.0 / norm_c
        featT = np.zeros((D, NLOC), np.float32)
        featT[:, :NLOC_REAL] = feat_c.T

        in_maps.append({
            "ginit": ginit, "g0s": g0s, "n2s": n2s, "invn": invn,
            "featT": featT, "w1": w1, "w2": w2,
            "b1": b1.reshape(D, 1), "b2": b2.reshape(D, 1),
            "gidx": H["gidx"][c], "selm": H["selm"][c],
        })

    nc = _build_program(H["calls"], H["bases"], H["totcols"], H["sumw"])

    from concourse.bass_utils import run_bass_kernel_spmd
    try:
        import ctypes
        import jax
        jax.devices()
        _lib = ctypes.CDLL("/opt/axon/libaxon_pjrt.so")
        _lib.axon_reset.restype = ctypes.c_int64
        _lib.axon_reset()
    except Exception:
        pass
    trace = os.environ.get("KERNEL_TRACE", "0") == "1"
    if trace:
        try:
            sys.path.insert(0, os.path.dirname(os.path.abspath(__file__)) + "/dev")
            import prof_util
            prof_util.install()
        except Exception:
            trace = False
    res = run_bass_kernel_spmd(nc, in_maps, core_ids=list(range(NC_N)), trace=trace)
    LAST_EXEC_NS = res.exec_time_ns

    rst_full = np.zeros((N_NODES, D), np.float32)
    r_full = np.zeros((N_NODES, D), np.float32)
    for c in range(NC_N):
        lo = c * NLOC_REAL
        p = perms[c]
        rst_full[lo + p] = res.results[c]["rst_out"][:NLOC_REAL]
        r_full[lo + p] = res.results[c]["r_out"][:NLOC_REAL]
    return rst_full, r_full
